# revision 55
# baseline (speedup 1.0000x reference)
"""NT-Xent / InfoNCE contrastive loss (SimCLR) on 8 TRN2 NeuronCores.

Problem: features [8192, 1024] f32.
  f = features / ||features||_row
  sim = f @ f.T / 0.07
  pos_i = sim[i, (i + 4096) mod 8192]
  denom_i = logsumexp_j!=i sim[i, j]
  loss = mean(denom - pos)

Sharding: row-parallel, SPMD-identical program. Core k owns rows
[1024k, 1024k+1024). Each core receives the full feature matrix rolled by
-1024k rows, pre-transposed to d-major and quantized to fp8 e4m3 on the host
(pure layout/dtype conversion): x8[p, s, j] = x[j, 128s+p].

Device pipeline (per core), all big matmuls fp8 DoubleRow (2 k-slices/instr,
0.5 cycles/row):
  1. Norms: [128,128] raw self-blocks <x8_j, x8_j> on PE (8 per PSUM tile),
     diagonals extracted with a tiled-eye mask multiply + segmented 3D
     reduce (DVE) -> n^2 for all 8192 gathered rows in compact [128, 64]
     layout (partition = j mod 128). Tile 0 (local rows) runs up front; tiles
     1-7 are pipelined into the gram loop.
  2. Scales: U = invT/(32 n) via ACT Ln/Exp chunks; local scales 32/n are
     PE-transposed to [8, 128] and broadcast across partitions with one-hot
     selector matmuls (no DMA on this chain); f8 = x8_local * bcast
     (DVE/GPSIMD split) is the normalized local operand.
  3. Gram blocks G[j-block 128, i 1024] = <x8_j raw, f8_i> accumulated over 4
     k-slice pairs in PSUM; positive-pair diagonal extracted by mask-multiply
     + reduce; exp on ACT with per-partition scale U[:, jb] (handles 1/n_j)
     and bias -ln 128, output straight to fp8 e5m2. The self-similarity
     diagonal is NOT masked; its (large, in-range) exp'd e5m2 values are
     extracted and subtracted from the row sums afterwards, which cancels
     exactly.
  4. Row sums S_i: ones-matmul partition reduction over the exp'd fp8 tiles,
     accumulated in PSUM across all 64 blocks; diagonal extract -> per-row
     sums; denom = Ln(128*(S - selfexp)); loss rows = denom - invT*pos.

Dummy warmup matmuls keep the tensor engine's frequency ramp (cost model
p-state) warm across the startup phase. DMA order puts the local x8 chunk
first so the norm chain starts as early as possible.

Host: loss = sum(all per-row losses) / 8192.
"""

import math
import sys

import numpy as np

try:  # concourse is normally on sys.path via the site config
    import concourse  # noqa: F401
except ImportError:  # pragma: no cover
    for _p in ("/opt/trn_rl_repo", "/root/.axon_site/_ro/trn_rl_repo"):
        if _p not in sys.path:
            sys.path.insert(0, _p)

import ml_dtypes

N = 8192
D = 1024
P = 128
NCORES = 8
ROWS = N // NCORES  # 1024 local rows per core
KT = D // P  # 8 contraction slices of 128
S2 = KT // 2  # 4 DoubleRow k-slice pairs
JB = N // P  # 64 gathered column blocks of 128
TEMPERATURE = 0.07
INVT = 1.0 / TEMPERATURE
SCALE = 32.0  # fp8 operand scale: f8 = 32 * x / ||x||
EXPC = 128.0  # exp values scaled by 1/EXPC to stay inside fp8 e5m2 range
EXP_BIAS = -math.log(EXPC)
U_BIAS = math.log(INVT / SCALE)  # U = exp(-0.5 ln n2 + U_BIAS) = invT/(32 n)
U32_BIAS = math.log(SCALE)  # u32 = exp(-0.5 ln n2 + ln 32) = 32/n

ACT_SET = "natural_log_exp_and_others"  # contains exp, ln, copy

_cache = {}


def _build_program():
    import concourse.bacc as bacc
    import concourse.mybir as mybir
    from concourse import tile

    f32 = mybir.dt.float32
    bf16 = mybir.dt.bfloat16
    f8e4 = mybir.dt.float8e4
    f8e5 = mybir.dt.float8e5
    AF = mybir.ActivationFunctionType
    AX = mybir.AxisListType
    ALU = mybir.AluOpType
    DR = mybir.MatmulPerfMode.DoubleRow

    # Pin every activation to one LUT set so the table-load pass emits a
    # single load instead of thrashing between per-function default sets.
    orig_tables = bacc.get_activation_tables

    def pinned_tables(arch):
        return {
            name: (funcs if name == ACT_SET else set())
            for name, funcs in orig_tables(arch).items()
        }

    bacc.get_activation_tables = pinned_tables
    try:
        nc = bacc.Bacc(
            "TRN2",
            target_bir_lowering=False,
            debug=False,
            num_devices=NCORES,
        )
        # x8[p, s, j] = fp8(x[j, 128 s + p]) -- host-transposed + quantized.
        x8d = nc.declare_dram_parameter("x8", [P, KT, N], f8e4, isOutput=False)
        # Tiled eye: eyeT[p, q] = (q % 128 == p), 8 copies of eye128.
        eyeT = nc.declare_dram_parameter("eyeT", [P, KT * P], bf16, isOutput=False)
        eye1 = nc.declare_dram_parameter("eye1", [P, P], f32, isOutput=False)
        ones8 = nc.declare_dram_parameter("ones8", [P, 2 * P], f8e5, isOutput=False)
        # onesb[k, 128c+q] = (k == c): one-hot row selectors for the scale
        # broadcast matmuls.
        onesb = nc.declare_dram_parameter("onesb", [KT, KT * P], bf16, isOutput=False)
        out = nc.declare_dram_parameter("out", [P, KT], f32, isOutput=True)

        with tile.TileContext(nc) as tc:
            with (
                tc.tile_pool(name="big", bufs=1) as big,
                tc.tile_pool(name="work", bufs=3) as work,
                tc.tile_pool(name="small", bufs=4) as small,
                tc.tile_pool(name="ep", bufs=4) as ep,
                tc.tile_pool(name="pp", space="PSUM", bufs=1) as pp,
            ):
                # ---- local x8 chunk wins the DMA queue; consts follow ----
                x8 = big.tile([P, KT, N], f8e4, tag="x8")
                nc.sync.dma_start(x8[:, :, 0:1024], x8d[:, :, 0:1024])

                ones8_sb = big.tile([P, 2, P], f8e5, tag="ones8")
                nc.sync.dma_start(ones8_sb[:], ones8[:])
                onesb_sb = big.tile([KT, KT * P], bf16, tag="onesb")
                nc.sync.dma_start(onesb_sb[:], onesb[:])
                eye1_sb = big.tile([P, P], f32, tag="eye1")
                nc.sync.dma_start(eye1_sb[:], eye1[:])
                eyeT_sb = big.tile([P, KT * P], bf16, tag="eyeT")
                nc.sync.dma_start(eyeT_sb[:], eyeT[:])

                for j0, j1 in ((1024, 3072), (3072, 5632), (5632, 8192)):
                    nc.sync.dma_start(x8[:, :, j0:j1], x8d[:, :, j0:j1])

                # ---- bias constants (activation bias must be an AP) ----
                b_U = small.tile([P, 1], f32, tag="bU", name="bU")
                nc.vector.memset(b_U[:], U_BIAS)
                b_u32 = small.tile([P, 1], f32, tag="bu32", name="bu32")
                nc.vector.memset(b_u32[:], U32_BIAS)
                b_exp = small.tile([P, 1], f32, tag="bexp", name="bexp")
                nc.vector.memset(b_exp[:], EXP_BIAS)

                # ---- PE p-state warmup: dummy matmuls on a memset tile (no
                # DMA dependency) so the tensor engine's frequency ramp is
                # warm when the first x8 chunk lands ----
                wsrc = small.tile([P, 2, P], f8e5, tag="wsrc", name="wsrc")
                nc.vector.memset(wsrc[:], 0)
                warm = pp.tile([P, 1024], f32, tag="g", bufs=3, name="warm")
                for w in range(60):
                    nc.tensor.matmul(
                        warm[:, 0:P],
                        wsrc[:],
                        wsrc[:, :, 0:P],
                        start=True,
                        stop=True,
                        perf_mode=DR,
                    )

                n2 = big.tile([P, JB], f32, tag="n2")
                U = big.tile([P, JB], f32, tag="U")

                def diag_tile(t, tag, tag_bufs):
                    # 8 raw self-blocks <x8_j, x8_j>, j in [1024t, 1024(t+1)),
                    # packed side by side; mask-mul (GPSIMD) + segmented
                    # reduce (DVE) extracts the diagonal as n2[:, 8t:8t+8].
                    dg = pp.tile([P, 1024], f32, tag=tag, bufs=tag_bufs, name="dg")
                    for b8 in range(8):
                        b = 8 * t + b8
                        for s in range(S2):
                            nc.tensor.matmul(
                                dg[:, P * b8 : P * (b8 + 1)],
                                x8[:, 2 * s : 2 * s + 2, P * b : P * (b + 1)],
                                x8[:, 2 * s : 2 * s + 2, P * b : P * (b + 1)],
                                start=(s == 0),
                                stop=(s == S2 - 1),
                                perf_mode=DR,
                            )
                    dgm = work.tile([P, 1024], f32, tag="dgm", name="dgm")
                    nc.vector.tensor_mul(dgm[:], dg[:], eyeT_sb[:])
                    nc.vector.reduce_sum(
                        n2[:, 8 * t : 8 * t + 8],
                        dgm[:].rearrange("p (a b) -> p a b", a=8),
                        axis=AX.X,
                    )

                def u_chunk(t0, t1):
                    # U[:, 8 t0 : 8 t1] = invT / (32 n) for diag tiles [t0, t1)
                    w = 8 * (t1 - t0)
                    tln = small.tile([P, 3 * KT], f32, tag="tln", bufs=2, name="tln")
                    nc.scalar.activation(
                        tln[:, 0:w], n2[:, 8 * t0 : 8 * t1], AF.Ln
                    )
                    nc.scalar.activation(
                        U[:, 8 * t0 : 8 * t1],
                        tln[:, 0:w],
                        AF.Exp,
                        scale=-0.5,
                        bias=b_U[:, 0:1],
                    )
                    return tln

                # ---- phase 1: local norms + scales ----
                diag_tile(0, "s", 1)
                tln0 = u_chunk(0, 1)
                u32c = small.tile([P, KT], f32, tag="u32c", name="u32c")
                nc.scalar.activation(
                    u32c[:], tln0[:, 0:KT], AF.Exp, scale=-0.5, bias=b_u32[:, 0:1]
                )
                # Broadcast the local scales along partitions without touching
                # the (contended) DMA queue: PE-transpose the compact [128, 8]
                # scales to [8, 128], copy to SBUF, then replicate each row
                # with a K=1 ones-matmul.
                pt = pp.tile([P, 1024], f32, tag="g", bufs=3, name="pt")
                nc.tensor.transpose(pt[0:KT, 0:P], u32c[:], eye1_sb[:])
                uT = small.tile([KT, P], bf16, tag="uT", name="uT")
                nc.vector.tensor_copy(uT[:], pt[0:KT, 0:P])
                # ubc[:, 128c:128c+128] = uT[c, :] via one-hot-row selector:
                # lhsT[k, p] = (k == c), rhs = uT -> out[p, q] = uT[c, q].
                ubc = pp.tile([P, 1024], f32, tag="g", bufs=3, name="ubc")
                for c in range(KT):
                    nc.tensor.matmul(
                        ubc[:, P * c : P * (c + 1)],
                        onesb_sb[:, P * c : P * (c + 1)],
                        uT[:],
                        start=True,
                        stop=True,
                    )

                # ---- phase 2: normalized local operand; the multiply stream
                # is the critical startup path, so split it DVE/Pool with the
                # SBUF staging copy on the (idle) scalar engine ----
                ubc_sb = work.tile([P, 1024], bf16, tag="ubcsb", name="ubcsb")
                nc.scalar.activation(ubc_sb[:], ubc[:], AF.Copy)
                f8 = big.tile([P, KT, ROWS], f8e4, tag="f8")
                for k in range(5):
                    nc.vector.tensor_mul(f8[:, k, :], x8[:, k, 0:ROWS], ubc[:])
                for k in range(5, KT):
                    nc.gpsimd.tensor_mul(f8[:, k, :], x8[:, k, 0:ROWS], ubc_sb[:])

                # keep the tensor engine's frequency ramp warm while DVE/Pool
                # produce f8 (PE would otherwise idle and drop p-state)
                for w in range(110):
                    nc.tensor.matmul(
                        warm[:, 0:P],
                        wsrc[:],
                        wsrc[:, :, 0:P],
                        start=True,
                        stop=True,
                        perf_mode=DR,
                    )

                # ---- phase 3: gram blocks + exp + row-sum accumulation ----
                spsum = pp.tile([P, 1024], f32, tag="s", bufs=1, name="spsum")
                posg = big.tile([P, KT], f32, tag="posg")
                selfexp = big.tile([P, KT], f32, tag="selfexp")
                e8s = {}

                def emit_srow(jp):
                    e8p = e8s.pop(jp)
                    for ic in range(2):
                        nc.tensor.matmul(
                            spsum[:, 512 * ic : 512 * (ic + 1)],
                            ones8_sb[:],
                            e8p[:, :, 512 * ic : 512 * (ic + 1)],
                            start=(jp == 0),
                            stop=(jp == 31),
                            perf_mode=DR,
                            skip_group_check=True,
                        )

                for jp in range(32):
                    # diag tiles 1-7 ride the gram loop's psum rotation; their
                    # U chunks (needed from gram block jb = 8t onwards) are
                    # batched to keep the ACT exp stream dense.
                    if jp in (2, 4, 6, 8, 10, 12, 14):
                        diag_tile(jp // 2, "g", 3)
                    if jp == 2:
                        u_chunk(1, 2)
                    elif jp == 6:
                        u_chunk(2, 4)
                    elif jp == 11:
                        u_chunk(4, 6)
                    elif jp == 15:
                        u_chunk(6, 8)
                    e8 = ep.tile([P, 2, ROWS], f8e5, tag="e8", name="e8")
                    e8s[jp] = e8
                    for h in range(2):
                        jb = 2 * jp + h
                        g = pp.tile([P, 1024], f32, tag="g", bufs=3, name="g")
                        for s in range(S2):
                            for ic in range(2):
                                nc.tensor.matmul(
                                    g[:, 512 * ic : 512 * (ic + 1)],
                                    x8[:, 2 * s : 2 * s + 2, jb * P : (jb + 1) * P],
                                    f8[:, 2 * s : 2 * s + 2, 512 * ic : 512 * (ic + 1)],
                                    start=(s == 0),
                                    stop=(s == S2 - 1),
                                    perf_mode=DR,
                                )
                        if 32 <= jb < 32 + KT:
                            c = jb - 32
                            pdump = work.tile([P, P], f32, tag="pdump", name="pdump")
                            nc.vector.tensor_mul(
                                pdump[:], g[:, c * P : (c + 1) * P], eyeT_sb[:, 0:P]
                            )
                            nc.vector.reduce_sum(
                                posg[:, c : c + 1], pdump[:], axis=AX.X
                            )
                        nc.scalar.activation(
                            e8[:, h, :],
                            g[:],
                            AF.Exp,
                            scale=U[:, jb : jb + 1],
                            bias=b_exp[:, 0:1],
                        )
                        if jb < KT:
                            # exp'd self-similarity diagonal, subtracted from
                            # the row sums below (exact cancellation).
                            sdump = work.tile([P, P], f32, tag="sdump", name="sdump")
                            nc.vector.tensor_mul(
                                sdump[:],
                                e8[:, h, jb * P : (jb + 1) * P],
                                eyeT_sb[:, 0:P],
                            )
                            nc.vector.reduce_sum(
                                selfexp[:, jb : jb + 1], sdump[:], axis=AX.X
                            )
                    if jp >= 2:
                        emit_srow(jp - 2)
                emit_srow(30)
                emit_srow(31)

                # ---- phase 4: per-row losses ----
                posU = small.tile([P, KT], f32, tag="posU", name="posU")
                nc.vector.tensor_mul(posU[:], posg[:], U[:, 32 : 32 + KT])
                sm = work.tile([P, 1024], f32, tag="sm", name="sm")
                nc.vector.tensor_mul(sm[:], spsum[:], eyeT_sb[:])
                sc = small.tile([P, KT], f32, tag="sc", name="sc")
                nc.vector.reduce_sum(
                    sc[:], sm[:].rearrange("p (a b) -> p a b", a=8), axis=AX.X
                )
                sc2 = small.tile([P, KT], f32, tag="sc2", name="sc2")
                nc.vector.tensor_sub(sc2[:], sc[:], selfexp[:])
                den = small.tile([P, KT], f32, tag="den", name="den")
                nc.scalar.activation(den[:], sc2[:], AF.Ln, scale=EXPC)
                out_sb = small.tile([P, KT], f32, tag="outsb", name="outsb")
                nc.vector.tensor_sub(out_sb[:], den[:], posU[:])
                nc.sync.dma_start(out[:], out_sb[:])

        nc.compile()
    finally:
        bacc.get_activation_tables = orig_tables
    return nc


def _get_program():
    if "nc" not in _cache:
        _cache["nc"] = _build_program()
    return _cache["nc"]


def _host_inputs(features: np.ndarray):
    features = np.ascontiguousarray(features, dtype=np.float32)
    eyeT = np.tile(np.eye(P, dtype=np.float32), (1, KT)).astype(ml_dtypes.bfloat16)
    eye1 = np.eye(P, dtype=np.float32)
    ones8 = np.ones([P, 2 * P], dtype=ml_dtypes.float8_e5m2)
    onesb = np.repeat(np.eye(KT, dtype=np.float32), P, axis=1).astype(
        ml_dtypes.bfloat16
    )
    in_maps = []
    for k in range(NCORES):
        xr = np.roll(features, -ROWS * k, axis=0)
        # x8[p, s, j] = fp8(xr[j, 128 s + p])
        x8 = np.ascontiguousarray(
            xr.T.reshape(KT, P, N).transpose(1, 0, 2)
        ).astype(ml_dtypes.float8_e4m3)
        in_maps.append(
            {
                "x8": x8,
                "eyeT": eyeT,
                "eye1": eye1,
                "ones8": ones8,
                "onesb": onesb,
            }
        )
    return in_maps


def kernel(features: np.ndarray, _trace: bool = False):
    from concourse.bass_utils import run_bass_kernel_spmd

    nc = _get_program()
    in_maps = _host_inputs(features)
    res = run_bass_kernel_spmd(
        nc,
        in_maps,
        core_ids=list(range(NCORES)),
        trace=_trace,
    )
    total = 0.0
    for r in res.results:
        total += r["out"].astype(np.float64).sum()
    loss = np.float32(total / N)
    if _trace:
        return loss, res
    return loss


# revision 56
# speedup vs baseline: 1.0039x; 1.0039x over previous
"""NT-Xent / InfoNCE contrastive loss (SimCLR) on 8 TRN2 NeuronCores.

Problem: features [8192, 1024] f32.
  f = features / ||features||_row
  sim = f @ f.T / 0.07
  pos_i = sim[i, (i + 4096) mod 8192]
  denom_i = logsumexp_j!=i sim[i, j]
  loss = mean(denom - pos)

Sharding: row-parallel, SPMD-identical program. Core k owns rows
[1024k, 1024k+1024). Each core receives the full feature matrix rolled by
-1024k rows, pre-transposed to d-major and quantized to fp8 e4m3 on the host
(pure layout/dtype conversion): x8[p, s, j] = x[j, 128s+p].

Device pipeline (per core), all big matmuls fp8 DoubleRow (2 k-slices/instr,
0.5 cycles/row):
  1. Norms: [128,128] raw self-blocks <x8_j, x8_j> on PE (8 per PSUM tile),
     diagonals extracted with a tiled-eye mask multiply + segmented 3D
     reduce (DVE) -> n^2 for all 8192 gathered rows in compact [128, 64]
     layout (partition = j mod 128). Tile 0 (local rows) runs up front; tiles
     1-7 are pipelined into the gram loop.
  2. Scales: U = invT/(32 n) via ACT Ln/Exp chunks; local scales 32/n are
     PE-transposed to [8, 128] and broadcast across partitions with one-hot
     selector matmuls (no DMA on this chain); f8 = x8_local * bcast
     (DVE/GPSIMD split) is the normalized local operand.
  3. Gram blocks G[j-block 128, i 1024] = <x8_j raw, f8_i> accumulated over 4
     k-slice pairs in PSUM; positive-pair diagonal extracted by mask-multiply
     + reduce; exp on ACT with per-partition scale U[:, jb] (handles 1/n_j)
     and bias -ln 128, output straight to fp8 e5m2. The self-similarity
     diagonal is NOT masked; its (large, in-range) exp'd e5m2 values are
     extracted and subtracted from the row sums afterwards, which cancels
     exactly.
  4. Row sums S_i: ones-matmul partition reduction over the exp'd fp8 tiles,
     accumulated in PSUM across all 64 blocks; diagonal extract -> per-row
     sums; denom = Ln(128*(S - selfexp)); loss rows = denom - invT*pos.

Dummy warmup matmuls keep the tensor engine's frequency ramp (cost model
p-state) warm across the startup phase. DMA order puts the local x8 chunk
first so the norm chain starts as early as possible.

Host: loss = sum(all per-row losses) / 8192.
"""

import math
import sys

import numpy as np

try:  # concourse is normally on sys.path via the site config
    import concourse  # noqa: F401
except ImportError:  # pragma: no cover
    for _p in ("/opt/trn_rl_repo", "/root/.axon_site/_ro/trn_rl_repo"):
        if _p not in sys.path:
            sys.path.insert(0, _p)

import ml_dtypes

N = 8192
D = 1024
P = 128
NCORES = 8
ROWS = N // NCORES  # 1024 local rows per core
KT = D // P  # 8 contraction slices of 128
S2 = KT // 2  # 4 DoubleRow k-slice pairs
JB = N // P  # 64 gathered column blocks of 128
TEMPERATURE = 0.07
INVT = 1.0 / TEMPERATURE
SCALE = 32.0  # fp8 operand scale: f8 = 32 * x / ||x||
EXPC = 128.0  # exp values scaled by 1/EXPC to stay inside fp8 e5m2 range
EXP_BIAS = -math.log(EXPC)
U_BIAS = math.log(INVT / SCALE)  # U = exp(-0.5 ln n2 + U_BIAS) = invT/(32 n)
U32_BIAS = math.log(SCALE)  # u32 = exp(-0.5 ln n2 + ln 32) = 32/n

ACT_SET = "natural_log_exp_and_others"  # contains exp, ln, copy

_cache = {}


def _build_program():
    import concourse.bacc as bacc
    import concourse.mybir as mybir
    from concourse import tile

    f32 = mybir.dt.float32
    bf16 = mybir.dt.bfloat16
    f8e4 = mybir.dt.float8e4
    f8e5 = mybir.dt.float8e5
    AF = mybir.ActivationFunctionType
    AX = mybir.AxisListType
    ALU = mybir.AluOpType
    DR = mybir.MatmulPerfMode.DoubleRow

    # Pin every activation to one LUT set so the table-load pass emits a
    # single load instead of thrashing between per-function default sets.
    orig_tables = bacc.get_activation_tables

    def pinned_tables(arch):
        return {
            name: (funcs if name == ACT_SET else set())
            for name, funcs in orig_tables(arch).items()
        }

    bacc.get_activation_tables = pinned_tables
    try:
        nc = bacc.Bacc(
            "TRN2",
            target_bir_lowering=False,
            debug=False,
            num_devices=NCORES,
        )
        # x8[p, s, j] = fp8(x[j, 128 s + p]) -- host-transposed + quantized.
        x8d = nc.declare_dram_parameter("x8", [P, KT, N], f8e4, isOutput=False)
        # Tiled eye: eyeT[p, q] = (q % 128 == p), 8 copies of eye128.
        eyeT = nc.declare_dram_parameter("eyeT", [P, KT * P], bf16, isOutput=False)
        eye1 = nc.declare_dram_parameter("eye1", [P, P], f32, isOutput=False)
        ones8 = nc.declare_dram_parameter("ones8", [P, 2 * P], f8e5, isOutput=False)
        # onesb[k, 128c+q] = (k == c): one-hot row selectors for the scale
        # broadcast matmuls.
        onesb = nc.declare_dram_parameter("onesb", [KT, KT * P], bf16, isOutput=False)
        out = nc.declare_dram_parameter("out", [P, KT], f32, isOutput=True)

        with tile.TileContext(nc) as tc:
            with (
                tc.tile_pool(name="big", bufs=1) as big,
                tc.tile_pool(name="work", bufs=3) as work,
                tc.tile_pool(name="small", bufs=4) as small,
                tc.tile_pool(name="ep", bufs=4) as ep,
                tc.tile_pool(name="pp", space="PSUM", bufs=1) as pp,
            ):
                # ---- local x8 chunk wins the DMA queue; consts follow ----
                x8 = big.tile([P, KT, N], f8e4, tag="x8")
                nc.sync.dma_start(x8[:, :, 0:1024], x8d[:, :, 0:1024])

                ones8_sb = big.tile([P, 2, P], f8e5, tag="ones8")
                nc.sync.dma_start(ones8_sb[:], ones8[:])
                onesb_sb = big.tile([KT, KT * P], bf16, tag="onesb")
                nc.sync.dma_start(onesb_sb[:], onesb[:])
                eye1_sb = big.tile([P, P], f32, tag="eye1")
                nc.sync.dma_start(eye1_sb[:], eye1[:])
                eyeT_sb = big.tile([P, KT * P], bf16, tag="eyeT")
                nc.sync.dma_start(eyeT_sb[:], eyeT[:])

                for j0, j1 in ((1024, 3072), (3072, 5632), (5632, 8192)):
                    nc.sync.dma_start(x8[:, :, j0:j1], x8d[:, :, j0:j1])

                # ---- bias constants (activation bias must be an AP) ----
                b_U = small.tile([P, 1], f32, tag="bU", name="bU")
                nc.vector.memset(b_U[:], U_BIAS)
                b_u32 = small.tile([P, 1], f32, tag="bu32", name="bu32")
                nc.vector.memset(b_u32[:], U32_BIAS)
                b_exp = small.tile([P, 1], f32, tag="bexp", name="bexp")
                nc.vector.memset(b_exp[:], EXP_BIAS)

                # ---- PE p-state warmup: dummy matmuls on a memset tile (no
                # DMA dependency) so the tensor engine's frequency ramp is
                # warm when the first x8 chunk lands ----
                wsrc = small.tile([P, 2, P], f8e5, tag="wsrc", name="wsrc")
                nc.vector.memset(wsrc[:], 0)
                warm = pp.tile([P, 1024], f32, tag="g", bufs=3, name="warm")
                for w in range(60):
                    nc.tensor.matmul(
                        warm[:, 0:P],
                        wsrc[:],
                        wsrc[:, :, 0:P],
                        start=True,
                        stop=True,
                        perf_mode=DR,
                    )

                n2 = big.tile([P, JB], f32, tag="n2")
                U = big.tile([P, JB], f32, tag="U")

                def diag_tile(t, tag, tag_bufs):
                    # 8 raw self-blocks <x8_j, x8_j>, j in [1024t, 1024(t+1)),
                    # packed side by side; mask-mul (GPSIMD) + segmented
                    # reduce (DVE) extracts the diagonal as n2[:, 8t:8t+8].
                    dg = pp.tile([P, 1024], f32, tag=tag, bufs=tag_bufs, name="dg")
                    for b8 in range(8):
                        b = 8 * t + b8
                        for s in range(S2):
                            nc.tensor.matmul(
                                dg[:, P * b8 : P * (b8 + 1)],
                                x8[:, 2 * s : 2 * s + 2, P * b : P * (b + 1)],
                                x8[:, 2 * s : 2 * s + 2, P * b : P * (b + 1)],
                                start=(s == 0),
                                stop=(s == S2 - 1),
                                perf_mode=DR,
                            )
                    dgm = work.tile([P, 1024], f32, tag="dgm", name="dgm")
                    nc.vector.tensor_mul(dgm[:], dg[:], eyeT_sb[:])
                    nc.vector.reduce_sum(
                        n2[:, 8 * t : 8 * t + 8],
                        dgm[:].rearrange("p (a b) -> p a b", a=8),
                        axis=AX.X,
                    )

                def u_chunk(t0, t1):
                    # U[:, 8 t0 : 8 t1] = invT / (32 n) for diag tiles [t0, t1)
                    w = 8 * (t1 - t0)
                    tln = small.tile([P, 3 * KT], f32, tag="tln", bufs=2, name="tln")
                    nc.scalar.activation(
                        tln[:, 0:w], n2[:, 8 * t0 : 8 * t1], AF.Ln
                    )
                    nc.scalar.activation(
                        U[:, 8 * t0 : 8 * t1],
                        tln[:, 0:w],
                        AF.Exp,
                        scale=-0.5,
                        bias=b_U[:, 0:1],
                    )
                    return tln

                # ---- phase 1: local norms + scales ----
                diag_tile(0, "s", 1)
                tln0 = u_chunk(0, 1)
                u32c = small.tile([P, KT], f32, tag="u32c", name="u32c")
                nc.scalar.activation(
                    u32c[:], tln0[:, 0:KT], AF.Exp, scale=-0.5, bias=b_u32[:, 0:1]
                )
                # Broadcast the local scales along partitions without touching
                # the (contended) DMA queue: PE-transpose the compact [128, 8]
                # scales to [8, 128], copy to SBUF, then replicate each row
                # with a K=1 ones-matmul.
                pt = pp.tile([P, 1024], f32, tag="g", bufs=3, name="pt")
                nc.tensor.transpose(pt[0:KT, 0:P], u32c[:], eye1_sb[:])
                uT = small.tile([KT, P], bf16, tag="uT", name="uT")
                nc.vector.tensor_copy(uT[:], pt[0:KT, 0:P])
                # ubc[:, 128c:128c+128] = uT[c, :] via one-hot-row selector:
                # lhsT[k, p] = (k == c), rhs = uT -> out[p, q] = uT[c, q].
                ubc = pp.tile([P, 1024], f32, tag="g", bufs=3, name="ubc")
                for c in range(KT):
                    nc.tensor.matmul(
                        ubc[:, P * c : P * (c + 1)],
                        onesb_sb[:, P * c : P * (c + 1)],
                        uT[:],
                        start=True,
                        stop=True,
                    )

                # ---- phase 2: normalized local operand; the multiply stream
                # is the critical startup path, so split it DVE/Pool with the
                # SBUF staging copy on the (idle) scalar engine ----
                ubc_sb = work.tile([P, 1024], bf16, tag="ubcsb", name="ubcsb")
                nc.scalar.activation(ubc_sb[:], ubc[:], AF.Copy)
                f8 = big.tile([P, KT, ROWS], f8e4, tag="f8")
                for k in range(5):
                    nc.vector.tensor_mul(f8[:, k, :], x8[:, k, 0:ROWS], ubc_sb[:])
                for k in range(5, KT):
                    nc.gpsimd.tensor_mul(f8[:, k, :], x8[:, k, 0:ROWS], ubc_sb[:])

                # keep the tensor engine's frequency ramp warm while DVE/Pool
                # produce f8 (PE would otherwise idle and drop p-state)
                for w in range(110):
                    nc.tensor.matmul(
                        warm[:, 0:P],
                        wsrc[:],
                        wsrc[:, :, 0:P],
                        start=True,
                        stop=True,
                        perf_mode=DR,
                    )

                # ---- phase 3: gram blocks + exp + row-sum accumulation ----
                spsum = pp.tile([P, 1024], f32, tag="s", bufs=1, name="spsum")
                posg = big.tile([P, KT], f32, tag="posg")
                selfexp = big.tile([P, KT], f32, tag="selfexp")
                e8s = {}

                def emit_srow(jp):
                    e8p = e8s.pop(jp)
                    for ic in range(2):
                        nc.tensor.matmul(
                            spsum[:, 512 * ic : 512 * (ic + 1)],
                            ones8_sb[:],
                            e8p[:, :, 512 * ic : 512 * (ic + 1)],
                            start=(jp == 0),
                            stop=(jp == 31),
                            perf_mode=DR,
                            skip_group_check=True,
                        )

                for jp in range(32):
                    # diag tiles 1-7 ride the gram loop's psum rotation; their
                    # U chunks (needed from gram block jb = 8t onwards) are
                    # batched to keep the ACT exp stream dense.
                    if jp in (2, 4, 6, 8, 10, 12, 14):
                        diag_tile(jp // 2, "g", 3)
                    if jp == 2:
                        u_chunk(1, 2)
                    elif jp == 6:
                        u_chunk(2, 4)
                    elif jp == 11:
                        u_chunk(4, 6)
                    elif jp == 15:
                        u_chunk(6, 8)
                    e8 = ep.tile([P, 2, ROWS], f8e5, tag="e8", name="e8")
                    e8s[jp] = e8
                    for h in range(2):
                        jb = 2 * jp + h
                        g = pp.tile([P, 1024], f32, tag="g", bufs=3, name="g")
                        for s in range(S2):
                            for ic in range(2):
                                nc.tensor.matmul(
                                    g[:, 512 * ic : 512 * (ic + 1)],
                                    x8[:, 2 * s : 2 * s + 2, jb * P : (jb + 1) * P],
                                    f8[:, 2 * s : 2 * s + 2, 512 * ic : 512 * (ic + 1)],
                                    start=(s == 0),
                                    stop=(s == S2 - 1),
                                    perf_mode=DR,
                                )
                        if 32 <= jb < 32 + KT:
                            c = jb - 32
                            pdump = work.tile([P, P], f32, tag="pdump", name="pdump")
                            nc.vector.tensor_mul(
                                pdump[:], g[:, c * P : (c + 1) * P], eyeT_sb[:, 0:P]
                            )
                            nc.vector.reduce_sum(
                                posg[:, c : c + 1], pdump[:], axis=AX.X
                            )
                        nc.scalar.activation(
                            e8[:, h, :],
                            g[:],
                            AF.Exp,
                            scale=U[:, jb : jb + 1],
                            bias=b_exp[:, 0:1],
                        )
                        if jb < KT:
                            # exp'd self-similarity diagonal, subtracted from
                            # the row sums below (exact cancellation).
                            sdump = work.tile([P, P], f32, tag="sdump", name="sdump")
                            nc.vector.tensor_mul(
                                sdump[:],
                                e8[:, h, jb * P : (jb + 1) * P],
                                eyeT_sb[:, 0:P],
                            )
                            nc.vector.reduce_sum(
                                selfexp[:, jb : jb + 1], sdump[:], axis=AX.X
                            )
                    if jp >= 2:
                        emit_srow(jp - 2)
                emit_srow(30)
                emit_srow(31)

                # ---- phase 4: per-row losses ----
                posU = small.tile([P, KT], f32, tag="posU", name="posU")
                nc.vector.tensor_mul(posU[:], posg[:], U[:, 32 : 32 + KT])
                sm = work.tile([P, 1024], f32, tag="sm", name="sm")
                nc.vector.tensor_mul(sm[:], spsum[:], eyeT_sb[:])
                sc = small.tile([P, KT], f32, tag="sc", name="sc")
                nc.vector.reduce_sum(
                    sc[:], sm[:].rearrange("p (a b) -> p a b", a=8), axis=AX.X
                )
                sc2 = small.tile([P, KT], f32, tag="sc2", name="sc2")
                nc.vector.tensor_sub(sc2[:], sc[:], selfexp[:])
                den = small.tile([P, KT], f32, tag="den", name="den")
                nc.scalar.activation(den[:], sc2[:], AF.Ln, scale=EXPC)
                out_sb = small.tile([P, KT], f32, tag="outsb", name="outsb")
                nc.vector.tensor_sub(out_sb[:], den[:], posU[:])
                nc.sync.dma_start(out[:], out_sb[:])

        nc.compile()
    finally:
        bacc.get_activation_tables = orig_tables
    return nc


def _get_program():
    if "nc" not in _cache:
        _cache["nc"] = _build_program()
    return _cache["nc"]


def _host_inputs(features: np.ndarray):
    features = np.ascontiguousarray(features, dtype=np.float32)
    eyeT = np.tile(np.eye(P, dtype=np.float32), (1, KT)).astype(ml_dtypes.bfloat16)
    eye1 = np.eye(P, dtype=np.float32)
    ones8 = np.ones([P, 2 * P], dtype=ml_dtypes.float8_e5m2)
    onesb = np.repeat(np.eye(KT, dtype=np.float32), P, axis=1).astype(
        ml_dtypes.bfloat16
    )
    in_maps = []
    for k in range(NCORES):
        xr = np.roll(features, -ROWS * k, axis=0)
        # x8[p, s, j] = fp8(xr[j, 128 s + p])
        x8 = np.ascontiguousarray(
            xr.T.reshape(KT, P, N).transpose(1, 0, 2)
        ).astype(ml_dtypes.float8_e4m3)
        in_maps.append(
            {
                "x8": x8,
                "eyeT": eyeT,
                "eye1": eye1,
                "ones8": ones8,
                "onesb": onesb,
            }
        )
    return in_maps


def kernel(features: np.ndarray, _trace: bool = False):
    from concourse.bass_utils import run_bass_kernel_spmd

    nc = _get_program()
    in_maps = _host_inputs(features)
    res = run_bass_kernel_spmd(
        nc,
        in_maps,
        core_ids=list(range(NCORES)),
        trace=_trace,
    )
    total = 0.0
    for r in res.results:
        total += r["out"].astype(np.float64).sum()
    loss = np.float32(total / N)
    if _trace:
        return loss, res
    return loss
